# revision 13
# baseline (speedup 1.0000x reference)
"""Causal MHA (B=12, T=1024, C=768, H=12) on 8 TRN2 cores — fp8/bf16 hybrid.

Sharding: core c owns batch c fully (unit A, 12 heads) plus 6 heads
(parity c%2) of batch 8+c//2 at full T (unit B); the two cores of a pair
each produce a partial output projection for the shared batch and the
host sums them (no on-chip collectives).

Precision plan (rel-err budget 2e-2, measured composite ~1.4e-2):
 - Q/K projections + scores: fp8 e4m3 DoubleRow (2 contraction blocks
   per pass at 0.5 cyc/row).  Q/K live in "quad" layout: 4 heads per
   [128, 2, T] tile, head-dim split 32+32 along the member axis, so the
   64-dim score contraction runs as a DoubleRow pair on 32 partitions.
   The W column permutation that produces this layout is free (host).
 - V projection: fp8 DoubleRow with x-residual and W-residual correction
   chains (residuals quantized in the same scale domain accumulate in
   the same PSUM group), output bf16.
 - Softmax: max-free exp (scores ~N(0,0.3)), Act engine, bf16 P tiles.
 - AV: P stationary / V moving => out[q, d] at 65 cycles per key block
   in bf16; denominator via ones column lands per-partition, so
   normalize is reciprocal + per-partition scale.  PE transpose flips
   [q, d] -> [d, q] for the output projection, absorbing the V bias in
   the PSUM->SBUF copy.
 - Output projection: plain bf16 matmuls.
Engines: Act does only exp (plus a few startup DMA issues); every
PSUM-reading consumer lives on DVE (GPSIMD cannot access PSUM on real
TRN2 even though CoreSim permits it); Pool/gpsimd handles the causal
affine_selects and DMA issue; matmul/transpose on PE; y stores on SP.

CoreSim core-0 estimate ~129.7us vs ~217.3us for the fp32r baseline.
"""

import sys

for _p in ("/opt/trn_rl_repo", "/opt/pypackages"):
    if _p not in sys.path:
        sys.path.insert(0, _p)

import numpy as np
import ml_dtypes

import concourse.bass as bass
import concourse.bacc as bacc
import concourse.tile as tile
from concourse import mybir
from concourse.bass_utils import run_bass_kernel_spmd

F32 = mybir.dt.float32
BF16 = mybir.dt.bfloat16
F8 = mybir.dt.float8e4
AF = mybir.ActivationFunctionType
DR = mybir.MatmulPerfMode.DoubleRow
ALU = mybir.AluOpType
E4 = ml_dtypes.float8_e4m3

B, T, C = 12, 1024, 768
NH, HD = 12, 64
QCH = 512          # query chunk (2 per unit)
NKB = T // 128     # 8 key blocks
N_CORES = 8

S_X = 32.0         # x fp8 scale
S_W = 2048.0       # weight fp8 scale
S_QK = 32.0        # q/k fp8 scale
A_QK = S_QK / (S_X * S_W)          # psum -> q/k scale
A_V = 1.0 / (S_X * S_W)            # psum -> v scale
A_S = 0.125 / (S_QK * S_QK)        # score psum -> softmax input scale


def build_nc():
    nc = bacc.Bacc("TRN2", target_bir_lowering=False, debug=False, num_devices=N_CORES)

    dram = {}

    def din(name, shape, dt=F8):
        dram[name] = nc.dram_tensor(name, shape, dt, kind="ExternalInput")
        return dram[name]

    # unit A (batch c): x + residual in cb-pair layout [128, pair, member, T]
    din("xa8", [128, 3, 2, T])
    din("xar8", [128, 3, 2, T])
    # unit B (half-heads of batch 8+c//2)
    din("xb8", [128, 3, 2, T])
    din("xbr8", [128, 3, 2, T])
    # A weights: [128, pair, member, out-cols]
    din("wq8", [128, 3, 2, 1024])    # permuted for triad layout
    din("wk8", [128, 3, 2, 1024])
    din("wv8", [128, 3, 2, C])       # natural
    din("wvr8", [128, 3, 2, C])
    din("wo16", [128, 6, C], BF16)   # natural, bf16, [feat-part, cb, out]
    # B weights (6 heads => 384 dims, quads padded to 512 for q/k)
    din("wqb8", [128, 3, 2, 512])
    din("wkb8", [128, 3, 2, 512])
    din("wvb8", [128, 3, 2, 384])
    din("wvbr8", [128, 3, 2, 384])
    din("wob16", [128, 3, C], BF16)
    # biases (f32): qk pre-scaled by S_QK, v-bias raw (added post-transpose)
    din("bqa", [128, 8], F32)
    din("bka", [128, 8], F32)
    din("bqb", [128, 4], F32)
    din("bkb", [128, 4], F32)
    din("bva", [128, 6], F32)        # [feat-pair layout, head-pair]
    din("bvb", [128, 3], F32)
    din("boa", [128, 6], F32)
    din("ident", [128, 128], BF16)

    ya = nc.dram_tensor("ya_t", [C, T], F32, kind="ExternalOutput")
    yb = nc.dram_tensor("yb_t", [C, T], BF16, kind="ExternalOutput")

    with tile.TileContext(nc) as tc:
        with (
            tc.tile_pool(name="persist", bufs=1) as persist,
            tc.tile_pool(name="wpool", bufs=1) as wpool,
            tc.tile_pool(name="act", bufs=1) as act,
            tc.tile_pool(name="pp", bufs=10) as ppool,
            tc.tile_pool(name="yn", bufs=12) as ynpool,
            tc.tile_pool(name="yout", bufs=4) as ypool,
            tc.tile_pool(name="ps_s", bufs=2, space="PSUM") as ps_s,
            tc.tile_pool(name="ps_av", bufs=1, space="PSUM") as ps_av,
            tc.tile_pool(name="ps_trden", bufs=1, space="PSUM") as ps_trden,
            tc.tile_pool(name="ps_proj", bufs=2, space="PSUM") as ps_proj,
        ):
            # ---- weight + x SBUF tiles -----------------------------------
            def load(name, shape, dt=F8, eng=None, tag=None):
                t = wpool.tile(shape, dt, name=name, tag=tag or name)
                (eng or nc.sync).dma_start(out=t, in_=dram[name][:])
                return t

            W = {}
            # staged loads: everything head-0 qc0 needs lands first
            xa8 = wpool.tile([128, 3, 2, T], F8, name="xa8", tag="xa8")
            nc.sync.dma_start(out=xa8[:, :, :, 0:QCH], in_=dram["xa8"][:, :, :, 0:QCH])
            W["wk8"] = wpool.tile([128, 3, 2, 1024], F8, name="wk8", tag="wk8")
            nc.sync.dma_start(out=W["wk8"][:, :, :, 0:256], in_=dram["wk8"][:, :, :, 0:256])
            W["wq8"] = wpool.tile([128, 3, 2, 1024], F8, name="wq8", tag="wq8")
            nc.sync.dma_start(out=W["wq8"][:, :, :, 0:256], in_=dram["wq8"][:, :, :, 0:256])
            nc.sync.dma_start(out=xa8[:, :, :, QCH:T], in_=dram["xa8"][:, :, :, QCH:T])
            W["wv8"] = load("wv8", [128, 3, 2, C], eng=nc.gpsimd)
            W["wvr8"] = load("wvr8", [128, 3, 2, C], eng=nc.gpsimd)
            xar8 = load("xar8", [128, 3, 2, T], eng=nc.gpsimd)
            bias_sb = {}
            for nm, w in (("bka", 8), ("bqa", 8), ("bqb", 4), ("bkb", 4),
                          ("bva", 6), ("bvb", 3), ("boa", 6)):
                t = persist.tile([128, w], F32, name=nm, tag=nm)
                nc.scalar.dma_start(out=t, in_=dram[nm][:])
                bias_sb[nm] = t
            ident_sb = persist.tile([128, 128], BF16, tag="ident")
            nc.scalar.dma_start(out=ident_sb, in_=dram["ident"][:])
            ones_sb = persist.tile([128, 1], BF16, tag="ones")
            nc.vector.memset(ones_sb, 1.0)
            nc.sync.dma_start(out=W["wk8"][:, :, :, 256:1024], in_=dram["wk8"][:, :, :, 256:1024])
            nc.sync.dma_start(out=W["wq8"][:, :, :, 256:1024], in_=dram["wq8"][:, :, :, 256:1024])

            # ---- psum suppliers ------------------------------------------
            def mk_proj_psum(width):
                return ps_proj.tile([128, width], F32, name="proj", tag="proj")

            def mk_sp_psum(width):
                # bulk phases only (attention not in flight): borrow the
                # score-pair banks for 2-deep projection pipelining
                sp = ps_s.tile([128, 2 * QCH], F32, name="sp", tag="sp")
                return sp[:, 0:width]

            _rr = [0]

            def mk_rr_psum(width):
                _rr[0] += 1
                return mk_proj_psum(width) if _rr[0] % 3 == 0 else mk_sp_psum(width)

            def mk_hook_psum(width):
                return mk_proj_psum(width)

            # ---- generic fp8 DR projection -------------------------------
            def proj_qk(wt, xt, xrt, dst, dblk, rc, bias, n_pairs=3, mk=None):
                """dst[quad][:, member, rc-cols] fp8; rc=None => full T."""
                w = QCH if rc is not None else T
                lo = rc * QCH if rc is not None else 0
                psum = (mk or mk_hook_psum)(w)
                chains = [(wt, xt)] if xrt is None else [(wt, xt), (wt, xrt)]
                n = len(chains) * n_pairs
                for sub in range(w // QCH):  # one chain per psum bank
                    i = 0
                    for wsrc, xsrc in chains:
                        for pr in range(n_pairs):
                            nc.tensor.matmul(
                                psum[:, sub * QCH:(sub + 1) * QCH],
                                wsrc[:, pr, :, dblk * 128:(dblk + 1) * 128],
                                xsrc[:, pr, :, lo + sub * QCH:lo + (sub + 1) * QCH],
                                start=(i == 0), stop=(i == n - 1), perf_mode=DR)
                            i += 1
                quad, m = dblk // 2, dblk % 2
                nc.vector.tensor_scalar(
                    out=dst[quad][:, m, lo:lo + w], in0=psum,
                    scalar1=A_QK, scalar2=bias[:, dblk:dblk + 1],
                    op0=ALU.mult, op1=ALU.add)

            def proj_v(wt, wrt, xt, xrt, v_tiles, rblk, half, nh6, mk=None):
                """v_tiles[rblk][:, half*6:+6, 0:64] bf16, 3-term fp8 DR."""
                ow = 384
                psum = (mk or mk_hook_psum)(ow)
                chains = [(wt, xt), (wt, xrt), (wrt, xt)]
                i, n = 0, 9
                for wsrc, xsrc in chains:
                    for pr in range(3):
                        nc.tensor.matmul(
                            psum,
                            xsrc[:, pr, :, rblk * 128:(rblk + 1) * 128],
                            wsrc[:, pr, :, half * ow:(half + 1) * ow] if nh6 == 2
                            else wsrc[:, pr, :, :],
                            start=(i == 0), stop=(i == n - 1), perf_mode=DR)
                        i += 1
                nc.vector.tensor_scalar(
                    out=v_tiles[rblk][:, half * 6:half * 6 + 6, :],
                    in0=psum.rearrange("p (h d) -> p h d", h=6),
                    scalar1=A_V, scalar2=None, op0=ALU.mult)

            # ---- attention for one unit ----------------------------------
            def attention(q_t, k_t, v_t, ao_t, bv_sb, n_heads, hooks):
                """Full-T causal attention, software-pipelined one head deep:
                head h's S+exp issue before head h-1's AV/normalize/transpose
                so the Act exp stream never waits on the PE's in-order queue.
                """
                def issue_s_exp(h, qc):
                    tri, b32 = h // 3, 32 * (h % 3)
                    nkb = 4 * (qc + 1)
                    p_sb = []
                    for kp in range(nkb // 2):
                        sp = ps_s.tile([128, 2 * QCH], F32, name="sp", tag="sp")
                        live = [max(0, (2 * kp + m) * 128 - qc * QCH) for m in range(2)]
                        # a small odd-member offset: compute the dead columns
                        # too (real but non-causal scores, never read by AV)
                        # so one wide exp can cover the pair
                        merge = live[0] == 0 and live[1] <= 128
                        for m in range(2):
                            kb = 2 * kp + m
                            d = 0 if merge else live[m]
                            nc.tensor.matmul(
                                sp[:, m * QCH + d:(m + 1) * QCH],
                                k_t[tri][b32:b32 + 32, :, kb * 128:(kb + 1) * 128],
                                q_t[tri][b32:b32 + 32, :, qc * QCH + d:(qc + 1) * QCH],
                                start=True, stop=True, perf_mode=DR)
                        pt = ppool.tile([128, 2 * QCH], BF16, name="p", tag="p")
                        if merge:
                            nc.scalar.activation(
                                out=pt[:, 0:2 * QCH], in_=sp[:, 0:2 * QCH],
                                func=AF.Exp, scale=A_S)
                        else:
                            for m in range(2):
                                nc.scalar.activation(
                                    out=pt[:, m * QCH + live[m]:(m + 1) * QCH],
                                    in_=sp[:, m * QCH + live[m]:(m + 1) * QCH],
                                    func=AF.Exp, scale=A_S)
                        for m in range(2):
                            kb = 2 * kp + m
                            off = kb * 128 - qc * QCH
                            if off >= 0:
                                w = min(QCH - off, 128)
                                nc.gpsimd.affine_select(
                                    out=pt[:, m * QCH + off:m * QCH + off + w],
                                    in_=pt[:, m * QCH + off:m * QCH + off + w],
                                    compare_op=ALU.is_ge, fill=0.0,
                                    base=0, pattern=[[1, w]],
                                    channel_multiplier=-1)
                        p_sb.append(pt)
                    return p_sb

                tr_state = {}

                def issue_avnorm(h, qc, p_sb):
                    hp = h // 2
                    if h % 2 == 0:
                        tr_state[0] = ps_trden.tile([128, QCH], BF16, name="tr", tag="tr")
                    tr_ps = tr_state[0]
                    av = ps_av.tile([128, 4, HD + 1], F32, name="av", tag="av")
                    for qb in range(4):
                        qg = qc * 4 + qb

                        def pslice(kb, qb=qb):
                            return p_sb[kb // 2][:, (kb % 2) * QCH + qb * 128:(kb % 2) * QCH + (qb + 1) * 128]

                        for kb in range(qg + 1):
                            nc.tensor.matmul(
                                av[:, qb, 0:HD], pslice(kb), v_t[kb][:, h, :],
                                start=(kb == 0), stop=(kb == qg))
                        for kb in range(qg + 1):
                            nc.tensor.matmul(
                                av[:, qb, HD:HD + 1], pslice(kb), ones_sb,
                                start=(kb == 0), stop=(kb == qg))
                    rbr = ynpool.tile([128, 4], F32, name="rbr", tag="rbr")
                    with nc.allow_low_precision(reason="softmax denom"):
                        nc.vector.reciprocal(out=rbr, in_=av[:, :, HD])
                    ynt = ynpool.tile([128, 4, HD], BF16, name="yn", tag="yn")
                    nc.vector.tensor_mul(
                        out=ynt, in0=av[:, :, 0:HD],
                        in1=rbr[:].to_broadcast([128, 4, HD]))
                    for qb in range(4):
                        nc.tensor.transpose(
                            tr_ps[(h % 2) * 64:(h % 2) * 64 + 64, qb * 128:(qb + 1) * 128],
                            ynt[:, qb, :], ident_sb)
                    if h % 2 == 1:
                        nc.vector.tensor_scalar(
                            out=ao_t[hp][:, qc * QCH:(qc + 1) * QCH], in0=tr_ps,
                            scalar1=bv_sb[:, hp:hp + 1], scalar2=None, op0=ALU.add)

                for qc in range(2):
                    hk = list(hooks.get(qc, []))
                    pending = None
                    for h in range(n_heads):
                        nfire = -(-len(hk) // (n_heads - h)) if hk else 0
                        for _ in range(nfire):
                            if hk:
                                hk.pop(0)()
                        p_sb = issue_s_exp(h, qc)
                        if pending is not None:
                            issue_avnorm(*pending)
                        pending = (h, qc, p_sb)
                    if pending is not None:
                        issue_avnorm(*pending)

            # ---- output projection (plain bf16) --------------------------
            def out_proj(wo_sb, ao_t, y_dram, rc, ncb, bias, dblk, mk=None):
                psum = (mk or mk_hook_psum)(QCH)
                for cb in range(ncb):
                    nc.tensor.matmul(
                        psum,
                        wo_sb[:, cb, dblk * 128:(dblk + 1) * 128],
                        ao_t[cb][:, rc * QCH:(rc + 1) * QCH],
                        start=(cb == 0), stop=(cb == ncb - 1))
                y_sb = ypool.tile([128, QCH], F32 if bias is not None else BF16,
                                  name="y", tag="ya" if bias is not None else "yb")
                if bias is not None:
                    nc.vector.tensor_scalar(
                        out=y_sb, in0=psum, scalar1=bias[:, dblk:dblk + 1],
                        scalar2=None, op0=ALU.add)
                else:
                    nc.vector.tensor_copy(out=y_sb, in_=psum)
                (nc.sync if dblk % 2 == 0 else nc.gpsimd).dma_start(
                    out=y_dram[dblk * 128:(dblk + 1) * 128, rc * QCH:(rc + 1) * QCH],
                    in_=y_sb)

            # ================== unit A =====================================
            k_t = [act.tile([128, 2, T], F8, name=f"k{j}", tag=f"k{j}") for j in range(4)]
            q_t = [act.tile([128, 2, T], F8, name=f"q{j}", tag=f"q{j}") for j in range(4)]
            v_t = [act.tile([128, NH, HD], BF16, name=f"v{kb}", tag=f"v{kb}")
                   for kb in range(NKB)]
            ao_t = [act.tile([128, T], BF16, name=f"ao{i}", tag=f"ao{i}") for i in range(6)]

            # minimal upfront set: only the exp-gating K/Q psums (2-deep in
            # the idle score banks; V rides the first hooks on ps_proj)
            for dblk in (0, 1):
                proj_qk(W["wk8"], xa8, None, k_t, dblk, 0, bias_sb["bka"], mk=mk_sp_psum)
            for dblk in (0, 1):
                proj_qk(W["wq8"], xa8, None, q_t, dblk, 0, bias_sb["bqa"], mk=mk_sp_psum)

            # (attention A is issued after hooksA[1] is populated below)
            # prefetch B inputs + weights (DMA queues run ahead of compute)
            xb8 = load("xb8", [128, 3, 2, T], eng=nc.gpsimd)
            xbr8 = load("xbr8", [128, 3, 2, T], eng=nc.gpsimd)
            W["wkb8"] = load("wkb8", [128, 3, 2, 512])
            W["wqb8"] = load("wqb8", [128, 3, 2, 512])
            W["wvb8"] = load("wvb8", [128, 3, 2, 384], eng=nc.gpsimd)
            W["wvbr8"] = load("wvbr8", [128, 3, 2, 384], eng=nc.gpsimd)
            W["wo16"] = load("wo16", [128, 6, C], BF16)
            W["wob16"] = load("wob16", [128, 3, C], BF16)

            # hooks: project remaining A quads during qc0 (rc-major so quad
            # j's qc0 columns are issued before its heads attend)
            def mk_qk(wt, xt, dst, dblk, rc, bias):
                return lambda: proj_qk(wt, xt, None, dst, dblk, rc, bias)

            def mk_v(wt, wrt, xt, xrt, dst, kb, half, nh6):
                return lambda: proj_v(wt, wrt, xt, xrt, dst, kb, half, nh6)

            qc0_hooks = []
            for kb in range(4):  # V for h0's AV, first thing
                qc0_hooks.append(mk_v(W["wv8"], W["wvr8"], xa8, xar8, v_t, kb, 0, 2))
            for d in (2, 3):  # tri1 needed from hp1
                qc0_hooks.append(mk_qk(W["wk8"], xa8, k_t, d, 0, bias_sb["bka"]))
                qc0_hooks.append(mk_qk(W["wq8"], xa8, q_t, d, 0, bias_sb["bqa"]))
            for kb in range(4):  # half1 V needed from hp3
                qc0_hooks.append(mk_v(W["wv8"], W["wvr8"], xa8, xar8, v_t, kb, 1, 2))
            for d in (4, 5, 6, 7):
                qc0_hooks.append(mk_qk(W["wk8"], xa8, k_t, d, 0, bias_sb["bka"]))
                qc0_hooks.append(mk_qk(W["wq8"], xa8, q_t, d, 0, bias_sb["bqa"]))
            for kb in range(4, NKB):
                qc0_hooks.append(mk_v(W["wv8"], W["wvr8"], xa8, xar8, v_t, kb, 0, 2))
            for d in (0, 1, 2, 3):  # rc1 for early qc1 heads
                qc0_hooks.append(mk_qk(W["wk8"], xa8, k_t, d, 1, bias_sb["bka"]))
                qc0_hooks.append(mk_qk(W["wq8"], xa8, q_t, d, 1, bias_sb["bqa"]))
            qc1_hooks = []
            for d in (4, 5, 6, 7):
                qc1_hooks.append(mk_qk(W["wk8"], xa8, k_t, d, 1, bias_sb["bka"]))
                qc1_hooks.append(mk_qk(W["wq8"], xa8, q_t, d, 1, bias_sb["bqa"]))
            for kb in range(4, NKB):
                qc1_hooks.append(mk_v(W["wv8"], W["wvr8"], xa8, xar8, v_t, kb, 1, 2))
            hooksA = {0: qc0_hooks, 1: qc1_hooks}

            # ================== unit B =====================================
            k2 = [act.tile([128, 2, T], F8, name=f"k2{j}", tag=f"k2{j}") for j in range(2)]
            q2 = [act.tile([128, 2, T], F8, name=f"q2{j}", tag=f"q2{j}") for j in range(2)]
            v2 = [act.tile([128, 6, HD], BF16, name=f"v2{kb}", tag=f"v2{kb}")
                  for kb in range(NKB)]
            ao2 = [act.tile([128, T], BF16, name=f"ao2{i}", tag=f"ao2{i}") for i in range(3)]

            hooksA[1] = hooksA[1] + (
                [lambda d=d, rc=rc: proj_qk(W["wkb8"], xb8, None, k2, d, rc, bias_sb["bkb"])
                 for rc in range(2) for d in range(4)] +
                [lambda d=d, rc=rc: proj_qk(W["wqb8"], xb8, None, q2, d, rc, bias_sb["bqb"])
                 for rc in range(2) for d in range(4)] +
                [mk_v(W["wvb8"], W["wvbr8"], xb8, xbr8, v2, kb, 0, 1) for kb in range(4)] +
                [lambda d=d: out_proj(W["wo16"], ao_t, ya, 0, 6, bias_sb["boa"], d)
                 for d in range(6)]
            )
            attention(q_t, k_t, v_t, ao_t, bias_sb["bva"], NH, hooksA)

            hooksB = {
                0: [mk_v(W["wvb8"], W["wvbr8"], xb8, xbr8, v2, kb, 0, 1)
                    for kb in range(4, NKB)],
                1: [lambda d=d: out_proj(W["wo16"], ao_t, ya, 1, 6, bias_sb["boa"], d)
                    for d in range(6)] +
                   [lambda d=d: out_proj(W["wob16"], ao2, yb, 0, 3, None, d)
                    for d in range(6)],
            }
            attention(q2, k2, v2, ao2, bias_sb["bvb"], 6, hooksB)
            for d in range(6):
                out_proj(W["wob16"], ao2, yb, 1, 3, None, d, mk=mk_rr_psum)

    nc.compile()
    return nc


_NC = None


def _get_nc():
    global _NC
    if _NC is None:
        _NC = build_nc()
    return _NC


def _q8(a, scale):
    return np.asarray(np.clip(a * scale, -240.0, 240.0), E4)


def _q8r(a, scale):
    """(hi, resid) e4m3 pair in the same scale domain."""
    s = np.clip(a * scale, -240.0, 240.0).astype(np.float32)
    hi = np.asarray(s, E4)
    r = np.asarray(s - hi.astype(np.float32), E4)
    return hi, r


def _pairpack(m):
    """[768, cols] -> [128, 3, 2, cols] (cb pairs along member axis)."""
    c, cols = m.shape
    return np.ascontiguousarray(m.reshape(3, 2, 128, cols).transpose(2, 0, 1, 3))


def _qk_perm(nh):
    """Column permutation producing triad layout: dblk 2j = heads 3j..3j+2
    dims 0-31 (+32 pad), dblk 2j+1 = dims 32-63 (+32 pad)."""
    idx, pad = [], []
    ntri = (nh + 2) // 3
    for j in range(ntri):
        for mhalf in range(2):
            for i in range(4):
                h = 3 * j + i
                if i < 3 and h < nh:
                    idx.extend(range(h * HD + 32 * mhalf, h * HD + 32 * mhalf + 32))
                    pad.extend([False] * 32)
                else:
                    idx.extend([0] * 32)  # pad slot
                    pad.extend([True] * 32)
    return np.array(idx), np.array(pad)


def make_in_maps(x, Wq, bq, Wk, bk, Wv, bv, Wo, bo):
    f = np.float32
    permA, padA = _qk_perm(12)
    permB6, padB6 = _qk_perm(6)  # within the 6-head slice

    def qk_pack(Wfull, perm, pad):
        Wt = Wfull.T.astype(f)  # [c, d]
        Wp = Wt[:, perm] * (~pad)[None, :]
        return _pairpack(_q8(Wp, S_W))

    def bias_qk(b, perm, pad, ndblk):
        bp = b[perm].astype(f) * S_QK * (~pad)
        return np.ascontiguousarray(bp.reshape(ndblk, 128).T)

    wq8 = qk_pack(Wq, permA, padA)
    wk8 = qk_pack(Wk, permA, padA)
    wv_hi, wv_r = _q8r(Wv.T.astype(f), S_W)
    wv8, wvr8 = _pairpack(wv_hi), _pairpack(wv_r)
    wo16 = np.ascontiguousarray(
        Wo.T.astype(ml_dtypes.bfloat16).reshape(6, 128, 768).transpose(1, 0, 2))
    bqa = bias_qk(bq, permA, padA, 8)
    bka = bias_qk(bk, permA, padA, 8)
    bva = np.ascontiguousarray(bv.astype(f).reshape(6, 128).T)
    boa = np.ascontiguousarray(bo.astype(f).reshape(6, 128).T)
    ident = np.eye(128, dtype=ml_dtypes.bfloat16)

    in_maps = []
    for c in range(N_CORES):
        par, bb = c % 2, 8 + c // 2
        hsl = slice(par * 6 * HD, (par * 6 + 6) * HD)
        xa_hi, xa_r = _q8r(x[c].T.astype(f), S_X)
        xb_hi, xb_r = _q8r(x[bb].T.astype(f), S_X)
        WqB, WkB, WvB = Wq[hsl], Wk[hsl], Wv[hsl]
        wvb_hi, wvb_r = _q8r(WvB.T.astype(f), S_W)
        wob16 = np.ascontiguousarray(
            Wo[:, hsl].T.astype(ml_dtypes.bfloat16).reshape(3, 128, 768).transpose(1, 0, 2))
        in_maps.append({
            "xa8": _pairpack(xa_hi), "xar8": _pairpack(xa_r),
            "xb8": _pairpack(xb_hi), "xbr8": _pairpack(xb_r),
            "wq8": wq8, "wk8": wk8, "wv8": wv8, "wvr8": wvr8, "wo16": wo16,
            "wqb8": qk_pack(WqB, permB6, padB6),
            "wkb8": qk_pack(WkB, permB6, padB6),
            "wvb8": _pairpack(wvb_hi), "wvbr8": _pairpack(wvb_r),
            "wob16": wob16,
            "bqa": bqa, "bka": bka,
            "bqb": bias_qk(bq[hsl], permB6, padB6, 4),
            "bkb": bias_qk(bk[hsl], permB6, padB6, 4),
            "bva": bva,
            "bvb": np.ascontiguousarray(bv[hsl].astype(f).reshape(3, 128).T),
            "boa": boa,
            "ident": ident,
        })
    return in_maps


def assemble(results, bo):
    out = np.empty((B, T, C), np.float32)
    for c in range(8):
        out[c] = results[c]["ya_t"].T
    for j in range(4):
        out[8 + j] = (results[2 * j]["yb_t"].astype(np.float32)
                      + results[2 * j + 1]["yb_t"].astype(np.float32)).T + bo
    return out


def kernel(**inputs):
    nc = _get_nc()
    in_maps = make_in_maps(**inputs)
    res = run_bass_kernel_spmd(nc, in_maps, list(range(N_CORES)))
    return assemble(res.results, inputs["bo"].astype(np.float32))


if __name__ == "__main__":
    rng = np.random.default_rng(0)
    inputs = {
        "x": rng.normal(size=(B, T, C)).astype(np.float32),
        **{k: (rng.normal(size=(C, C)) * 0.02).astype(np.float32)
           for k in ("Wq", "Wk", "Wv", "Wo")},
        **{k: (rng.normal(size=(C,)) * 0.02).astype(np.float32)
           for k in ("bq", "bk", "bv", "bo")},
    }
    out = kernel(**inputs)
    print(out.shape, out.dtype)


# revision 14
# speedup vs baseline: 1.0825x; 1.0825x over previous
"""Causal MHA (B=12, T=1024, C=768, H=12) on 8 TRN2 cores — fp8/bf16 hybrid.

Sharding: core c owns batch c fully (unit A, 12 heads) plus 6 heads
(parity c%2) of batch 8+c//2 at full T (unit B); the two cores of a pair
each produce a partial output projection for the shared batch and the
host sums them (no on-chip collectives).

Precision plan (rel-err budget 2e-2, measured composite ~1.4e-2):
 - Q/K projections + scores: fp8 e4m3 DoubleRow (2 contraction blocks
   per pass at 0.5 cyc/row).  Q/K live in "quad" layout: 4 heads per
   [128, 2, T] tile, head-dim split 32+32 along the member axis, so the
   64-dim score contraction runs as a DoubleRow pair on 32 partitions.
   The W column permutation that produces this layout is free (host).
 - V projection: fp8 DoubleRow with x-residual and W-residual correction
   chains (residuals quantized in the same scale domain accumulate in
   the same PSUM group), output bf16.
 - Softmax: max-free exp (scores ~N(0,0.3)), Act engine, bf16 P tiles.
 - AV: P stationary / V moving => out[q, d] at 65 cycles per key block
   in bf16; denominator via ones column lands per-partition, so
   normalize is reciprocal + per-partition scale.  PE transpose flips
   [q, d] -> [d, q] for the output projection, absorbing the V bias in
   the PSUM->SBUF copy.
 - Output projection: plain bf16 matmuls.
Engines: Act does only exp (plus a few startup DMA issues); every
PSUM-reading consumer lives on DVE (GPSIMD cannot access PSUM on real
TRN2 even though CoreSim permits it); Pool/gpsimd handles the causal
affine_selects and DMA issue; matmul/transpose on PE; y stores on SP.

CoreSim core-0 estimate ~129.3us vs ~217.3us for the fp32r baseline.
"""

import sys

for _p in ("/opt/trn_rl_repo", "/opt/pypackages"):
    if _p not in sys.path:
        sys.path.insert(0, _p)

import numpy as np
import ml_dtypes

import concourse.bass as bass
import concourse.bacc as bacc
import concourse.tile as tile
from concourse import mybir
from concourse.bass_utils import run_bass_kernel_spmd

F32 = mybir.dt.float32
BF16 = mybir.dt.bfloat16
F8 = mybir.dt.float8e4
AF = mybir.ActivationFunctionType
DR = mybir.MatmulPerfMode.DoubleRow
ALU = mybir.AluOpType
E4 = ml_dtypes.float8_e4m3

B, T, C = 12, 1024, 768
NH, HD = 12, 64
QCH = 512          # query chunk (2 per unit)
NKB = T // 128     # 8 key blocks
N_CORES = 8

S_X = 32.0         # x fp8 scale
S_W = 2048.0       # weight fp8 scale
S_QK = 32.0        # q/k fp8 scale
A_QK = S_QK / (S_X * S_W)          # psum -> q/k scale
A_V = 1.0 / (S_X * S_W)            # psum -> v scale
A_S = 0.125 / (S_QK * S_QK)        # score psum -> softmax input scale


def build_nc():
    nc = bacc.Bacc("TRN2", target_bir_lowering=False, debug=False, num_devices=N_CORES)

    dram = {}

    def din(name, shape, dt=F8):
        dram[name] = nc.dram_tensor(name, shape, dt, kind="ExternalInput")
        return dram[name]

    # unit A (batch c): x + residual in cb-pair layout [128, pair, member, T]
    din("xa8", [128, 3, 2, T])
    din("xar8", [128, 3, 2, T])
    # unit B (half-heads of batch 8+c//2)
    din("xb8", [128, 3, 2, T])
    din("xbr8", [128, 3, 2, T])
    # A weights: [128, pair, member, out-cols]
    din("wq8", [128, 3, 2, 1024])    # permuted for triad layout
    din("wk8", [128, 3, 2, 1024])
    din("wv8", [128, 3, 2, C])       # natural
    din("wvr8", [128, 3, 2, C])
    din("wo16", [128, 6, C], BF16)   # natural, bf16, [feat-part, cb, out]
    # B weights (6 heads => 384 dims, quads padded to 512 for q/k)
    din("wqb8", [128, 3, 2, 512])
    din("wkb8", [128, 3, 2, 512])
    din("wvb8", [128, 3, 2, 384])
    din("wvbr8", [128, 3, 2, 384])
    din("wob16", [128, 3, C], BF16)
    # biases (f32): qk pre-scaled by S_QK, v-bias raw (added post-transpose)
    din("bqa", [128, 8], F32)
    din("bka", [128, 8], F32)
    din("bqb", [128, 4], F32)
    din("bkb", [128, 4], F32)
    din("bva", [128, 6], F32)        # [feat-pair layout, head-pair]
    din("bvb", [128, 3], F32)
    din("boa", [128, 6], F32)
    din("ident", [128, 128], BF16)

    ya = nc.dram_tensor("ya_t", [C, T], F32, kind="ExternalOutput")
    yb = nc.dram_tensor("yb_t", [C, T], BF16, kind="ExternalOutput")

    with tile.TileContext(nc) as tc:
        with (
            tc.tile_pool(name="persist", bufs=1) as persist,
            tc.tile_pool(name="wpool", bufs=1) as wpool,
            tc.tile_pool(name="act", bufs=1) as act,
            tc.tile_pool(name="pp", bufs=10) as ppool,
            tc.tile_pool(name="yn", bufs=12) as ynpool,
            tc.tile_pool(name="yout", bufs=4) as ypool,
            tc.tile_pool(name="ps_s", bufs=2, space="PSUM") as ps_s,
            tc.tile_pool(name="ps_av", bufs=1, space="PSUM") as ps_av,
            tc.tile_pool(name="ps_trden", bufs=1, space="PSUM") as ps_trden,
            tc.tile_pool(name="ps_proj", bufs=2, space="PSUM") as ps_proj,
        ):
            # ---- weight + x SBUF tiles -----------------------------------
            def load(name, shape, dt=F8, eng=None, tag=None):
                t = wpool.tile(shape, dt, name=name, tag=tag or name)
                (eng or nc.sync).dma_start(out=t, in_=dram[name][:])
                return t

            W = {}
            # staged loads: everything head-0 qc0 needs lands first
            xa8 = wpool.tile([128, 3, 2, T], F8, name="xa8", tag="xa8")
            nc.sync.dma_start(out=xa8[:, :, :, 0:QCH], in_=dram["xa8"][:, :, :, 0:QCH])
            W["wk8"] = wpool.tile([128, 3, 2, 1024], F8, name="wk8", tag="wk8")
            nc.sync.dma_start(out=W["wk8"][:, :, :, 0:256], in_=dram["wk8"][:, :, :, 0:256])
            W["wq8"] = wpool.tile([128, 3, 2, 1024], F8, name="wq8", tag="wq8")
            nc.sync.dma_start(out=W["wq8"][:, :, :, 0:256], in_=dram["wq8"][:, :, :, 0:256])
            nc.sync.dma_start(out=xa8[:, :, :, QCH:T], in_=dram["xa8"][:, :, :, QCH:T])
            W["wv8"] = load("wv8", [128, 3, 2, C], eng=nc.gpsimd)
            W["wvr8"] = load("wvr8", [128, 3, 2, C], eng=nc.gpsimd)
            xar8 = load("xar8", [128, 3, 2, T], eng=nc.gpsimd)
            bias_sb = {}
            for nm, w in (("bka", 8), ("bqa", 8), ("bqb", 4), ("bkb", 4),
                          ("bva", 6), ("bvb", 3), ("boa", 6)):
                t = persist.tile([128, w], F32, name=nm, tag=nm)
                nc.scalar.dma_start(out=t, in_=dram[nm][:])
                bias_sb[nm] = t
            ident_sb = persist.tile([128, 128], BF16, tag="ident")
            nc.scalar.dma_start(out=ident_sb, in_=dram["ident"][:])
            ones_sb = persist.tile([128, 1], BF16, tag="ones")
            nc.vector.memset(ones_sb, 1.0)
            nc.sync.dma_start(out=W["wk8"][:, :, :, 256:1024], in_=dram["wk8"][:, :, :, 256:1024])
            nc.sync.dma_start(out=W["wq8"][:, :, :, 256:1024], in_=dram["wq8"][:, :, :, 256:1024])

            # ---- psum suppliers ------------------------------------------
            def mk_proj_psum(width):
                return ps_proj.tile([128, width], F32, name="proj", tag="proj")

            def mk_sp_psum(width):
                # bulk phases only (attention not in flight): borrow the
                # score-pair banks for 2-deep projection pipelining
                sp = ps_s.tile([128, 2 * QCH], F32, name="sp", tag="sp")
                return sp[:, 0:width]

            _rr = [0]

            def mk_rr_psum(width):
                _rr[0] += 1
                return mk_proj_psum(width) if _rr[0] % 3 == 0 else mk_sp_psum(width)

            def mk_hook_psum(width):
                return mk_proj_psum(width)

            # ---- generic fp8 DR projection -------------------------------
            def proj_qk(wt, xt, xrt, dst, dblk, rc, bias, n_pairs=3, mk=None):
                """dst[quad][:, member, rc-cols] fp8; rc=None => full T."""
                w = QCH if rc is not None else T
                lo = rc * QCH if rc is not None else 0
                psum = (mk or mk_hook_psum)(w)
                chains = [(wt, xt)] if xrt is None else [(wt, xt), (wt, xrt)]
                n = len(chains) * n_pairs
                for sub in range(w // QCH):  # one chain per psum bank
                    i = 0
                    for wsrc, xsrc in chains:
                        for pr in range(n_pairs):
                            nc.tensor.matmul(
                                psum[:, sub * QCH:(sub + 1) * QCH],
                                wsrc[:, pr, :, dblk * 128:(dblk + 1) * 128],
                                xsrc[:, pr, :, lo + sub * QCH:lo + (sub + 1) * QCH],
                                start=(i == 0), stop=(i == n - 1), perf_mode=DR)
                            i += 1
                quad, m = dblk // 2, dblk % 2
                nc.vector.tensor_scalar(
                    out=dst[quad][:, m, lo:lo + w], in0=psum,
                    scalar1=A_QK, scalar2=bias[:, dblk:dblk + 1],
                    op0=ALU.mult, op1=ALU.add)

            def proj_v(wt, wrt, xt, xrt, v_tiles, rblk, half, nh6, mk=None):
                """v_tiles[rblk][:, half*6:+6, 0:64] bf16, 3-term fp8 DR."""
                ow = 384
                psum = (mk or mk_hook_psum)(ow)
                chains = [(wt, xt), (wt, xrt), (wrt, xt)]
                i, n = 0, 9
                for wsrc, xsrc in chains:
                    for pr in range(3):
                        nc.tensor.matmul(
                            psum,
                            xsrc[:, pr, :, rblk * 128:(rblk + 1) * 128],
                            wsrc[:, pr, :, half * ow:(half + 1) * ow] if nh6 == 2
                            else wsrc[:, pr, :, :],
                            start=(i == 0), stop=(i == n - 1), perf_mode=DR)
                        i += 1
                nc.vector.tensor_scalar(
                    out=v_tiles[rblk][:, half * 6:half * 6 + 6, :],
                    in0=psum.rearrange("p (h d) -> p h d", h=6),
                    scalar1=A_V, scalar2=None, op0=ALU.mult)

            # ---- attention for one unit ----------------------------------
            def attention(q_t, k_t, v_t, ao_t, bv_sb, n_heads, hooks):
                """Full-T causal attention, software-pipelined one head deep:
                head h's S+exp issue before head h-1's AV/normalize/transpose
                so the Act exp stream never waits on the PE's in-order queue.
                """
                def issue_s_exp(h, qc):
                    tri, b32 = h // 3, 32 * (h % 3)
                    nkb = 4 * (qc + 1)
                    p_sb = []
                    for kp in range(nkb // 2):
                        sp = ps_s.tile([128, 2 * QCH], F32, name="sp", tag="sp")
                        live = [max(0, (2 * kp + m) * 128 - qc * QCH) for m in range(2)]
                        # a small odd-member offset: compute the dead columns
                        # too (real but non-causal scores, never read by AV)
                        # so one wide exp can cover the pair
                        merge = live[0] == 0 and live[1] <= 128
                        for m in range(2):
                            kb = 2 * kp + m
                            d = 0 if merge else live[m]
                            nc.tensor.matmul(
                                sp[:, m * QCH + d:(m + 1) * QCH],
                                k_t[tri][b32:b32 + 32, :, kb * 128:(kb + 1) * 128],
                                q_t[tri][b32:b32 + 32, :, qc * QCH + d:(qc + 1) * QCH],
                                start=True, stop=True, perf_mode=DR)
                        pt = ppool.tile([128, 2 * QCH], BF16, name="p", tag="p")
                        if merge:
                            nc.scalar.activation(
                                out=pt[:, 0:2 * QCH], in_=sp[:, 0:2 * QCH],
                                func=AF.Exp, scale=A_S)
                        else:
                            for m in range(2):
                                nc.scalar.activation(
                                    out=pt[:, m * QCH + live[m]:(m + 1) * QCH],
                                    in_=sp[:, m * QCH + live[m]:(m + 1) * QCH],
                                    func=AF.Exp, scale=A_S)
                        for m in range(2):
                            kb = 2 * kp + m
                            off = kb * 128 - qc * QCH
                            if off >= 0:
                                w = min(QCH - off, 128)
                                nc.gpsimd.affine_select(
                                    out=pt[:, m * QCH + off:m * QCH + off + w],
                                    in_=pt[:, m * QCH + off:m * QCH + off + w],
                                    compare_op=ALU.is_ge, fill=0.0,
                                    base=0, pattern=[[1, w]],
                                    channel_multiplier=-1)
                        p_sb.append(pt)
                    return p_sb

                tr_state = {}

                def issue_avnorm(h, qc, p_sb):
                    hp = h // 2
                    if h % 2 == 0:
                        tr_state[0] = ps_trden.tile([128, QCH], BF16, name="tr", tag="tr")
                    tr_ps = tr_state[0]
                    av = ps_av.tile([128, 4, HD + 1], F32, name="av", tag="av")
                    for qb in range(4):
                        qg = qc * 4 + qb

                        def pslice(kb, qb=qb):
                            return p_sb[kb // 2][:, (kb % 2) * QCH + qb * 128:(kb % 2) * QCH + (qb + 1) * 128]

                        for kb in range(qg + 1):
                            nc.tensor.matmul(
                                av[:, qb, 0:HD], pslice(kb), v_t[kb][:, h, :],
                                start=(kb == 0), stop=(kb == qg))
                        for kb in range(qg + 1):
                            nc.tensor.matmul(
                                av[:, qb, HD:HD + 1], pslice(kb), ones_sb,
                                start=(kb == 0), stop=(kb == qg))
                    rbr = ynpool.tile([128, 4], F32, name="rbr", tag="rbr")
                    with nc.allow_low_precision(reason="softmax denom"):
                        nc.vector.reciprocal(out=rbr, in_=av[:, :, HD])
                    ynt = ynpool.tile([128, 4, HD], BF16, name="yn", tag="yn")
                    nc.vector.tensor_mul(
                        out=ynt, in0=av[:, :, 0:HD],
                        in1=rbr[:].to_broadcast([128, 4, HD]))
                    for qb in range(4):
                        nc.tensor.transpose(
                            tr_ps[(h % 2) * 64:(h % 2) * 64 + 64, qb * 128:(qb + 1) * 128],
                            ynt[:, qb, :], ident_sb)
                    if h % 2 == 1:
                        nc.vector.tensor_scalar(
                            out=ao_t[hp][:, qc * QCH:(qc + 1) * QCH], in0=tr_ps,
                            scalar1=bv_sb[:, hp:hp + 1], scalar2=None, op0=ALU.add)

                for qc in range(2):
                    hk = list(hooks.get(qc, []))
                    pending = None
                    for h in range(n_heads):
                        nfire = -(-len(hk) // (n_heads - h)) if hk else 0
                        for _ in range(nfire):
                            if hk:
                                hk.pop(0)()
                        p_sb = issue_s_exp(h, qc)
                        if pending is not None:
                            issue_avnorm(*pending)
                        pending = (h, qc, p_sb)
                    if pending is not None:
                        issue_avnorm(*pending)

            # ---- output projection (plain bf16) --------------------------
            def out_proj(wo_sb, ao_t, y_dram, rc, ncb, bias, dblk, mk=None):
                psum = (mk or mk_hook_psum)(QCH)
                for cb in range(ncb):
                    nc.tensor.matmul(
                        psum,
                        wo_sb[:, cb, dblk * 128:(dblk + 1) * 128],
                        ao_t[cb][:, rc * QCH:(rc + 1) * QCH],
                        start=(cb == 0), stop=(cb == ncb - 1))
                y_sb = ypool.tile([128, QCH], F32 if bias is not None else BF16,
                                  name="y", tag="ya" if bias is not None else "yb")
                if bias is not None:
                    nc.vector.tensor_scalar(
                        out=y_sb, in0=psum, scalar1=bias[:, dblk:dblk + 1],
                        scalar2=None, op0=ALU.add)
                else:
                    nc.vector.tensor_copy(out=y_sb, in_=psum)
                (nc.sync if dblk % 2 == 0 else nc.gpsimd).dma_start(
                    out=y_dram[dblk * 128:(dblk + 1) * 128, rc * QCH:(rc + 1) * QCH],
                    in_=y_sb)

            # ================== unit A =====================================
            k_t = [act.tile([128, 2, T], F8, name=f"k{j}", tag=f"k{j}") for j in range(4)]
            q_t = [act.tile([128, 2, T], F8, name=f"q{j}", tag=f"q{j}") for j in range(4)]
            v_t = [act.tile([128, NH, HD], BF16, name=f"v{kb}", tag=f"v{kb}")
                   for kb in range(NKB)]
            ao_t = [act.tile([128, T], BF16, name=f"ao{i}", tag=f"ao{i}") for i in range(6)]

            # minimal upfront set: only the exp-gating K/Q psums (2-deep in
            # the idle score banks; V rides the first hooks on ps_proj)
            for dblk in (0, 1):
                proj_qk(W["wk8"], xa8, None, k_t, dblk, 0, bias_sb["bka"], mk=mk_sp_psum)
            for dblk in (0, 1):
                proj_qk(W["wq8"], xa8, None, q_t, dblk, 0, bias_sb["bqa"], mk=mk_sp_psum)

            # (attention A is issued after hooksA[1] is populated below)
            # prefetch B inputs + weights (DMA queues run ahead of compute)
            xb8 = load("xb8", [128, 3, 2, T], eng=nc.gpsimd)
            xbr8 = load("xbr8", [128, 3, 2, T], eng=nc.gpsimd)
            W["wkb8"] = load("wkb8", [128, 3, 2, 512])
            W["wqb8"] = load("wqb8", [128, 3, 2, 512])
            W["wvb8"] = load("wvb8", [128, 3, 2, 384], eng=nc.gpsimd)
            W["wvbr8"] = load("wvbr8", [128, 3, 2, 384], eng=nc.gpsimd)
            W["wo16"] = load("wo16", [128, 6, C], BF16)
            W["wob16"] = load("wob16", [128, 3, C], BF16)

            # hooks: project remaining A quads during qc0 (rc-major so quad
            # j's qc0 columns are issued before its heads attend)
            def mk_qk(wt, xt, dst, dblk, rc, bias):
                return lambda: proj_qk(wt, xt, None, dst, dblk, rc, bias)

            def mk_v(wt, wrt, xt, xrt, dst, kb, half, nh6):
                return lambda: proj_v(wt, wrt, xt, xrt, dst, kb, half, nh6)

            qc0_hooks = []
            for kb in range(4):  # V for h0's AV, first thing
                qc0_hooks.append(mk_v(W["wv8"], W["wvr8"], xa8, xar8, v_t, kb, 0, 2))
            for d in (2, 3):  # tri1 needed from hp1
                qc0_hooks.append(mk_qk(W["wk8"], xa8, k_t, d, 0, bias_sb["bka"]))
                qc0_hooks.append(mk_qk(W["wq8"], xa8, q_t, d, 0, bias_sb["bqa"]))
            for kb in range(4):  # half1 V needed from hp3
                qc0_hooks.append(mk_v(W["wv8"], W["wvr8"], xa8, xar8, v_t, kb, 1, 2))
            for d in (4, 5, 6, 7):
                qc0_hooks.append(mk_qk(W["wk8"], xa8, k_t, d, 0, bias_sb["bka"]))
                qc0_hooks.append(mk_qk(W["wq8"], xa8, q_t, d, 0, bias_sb["bqa"]))
            for kb in range(4, NKB):
                qc0_hooks.append(mk_v(W["wv8"], W["wvr8"], xa8, xar8, v_t, kb, 0, 2))
            for d in (0, 1, 2, 3):  # rc1 for early qc1 heads
                qc0_hooks.append(mk_qk(W["wk8"], xa8, k_t, d, 1, bias_sb["bka"]))
                qc0_hooks.append(mk_qk(W["wq8"], xa8, q_t, d, 1, bias_sb["bqa"]))
            qc1_hooks = []
            for d in (4, 5, 6, 7):
                qc1_hooks.append(mk_qk(W["wk8"], xa8, k_t, d, 1, bias_sb["bka"]))
                qc1_hooks.append(mk_qk(W["wq8"], xa8, q_t, d, 1, bias_sb["bqa"]))
            for kb in range(4, NKB):
                qc1_hooks.append(mk_v(W["wv8"], W["wvr8"], xa8, xar8, v_t, kb, 1, 2))
            hooksA = {0: qc0_hooks, 1: qc1_hooks}

            # ================== unit B =====================================
            k2 = [act.tile([128, 2, T], F8, name=f"k2{j}", tag=f"k2{j}") for j in range(2)]
            q2 = [act.tile([128, 2, T], F8, name=f"q2{j}", tag=f"q2{j}") for j in range(2)]
            v2 = [act.tile([128, 6, HD], BF16, name=f"v2{kb}", tag=f"v2{kb}")
                  for kb in range(NKB)]
            ao2 = [act.tile([128, T], BF16, name=f"ao2{i}", tag=f"ao2{i}") for i in range(3)]

            hooksA[1] = hooksA[1] + (
                [lambda d=d, rc=rc: proj_qk(W["wkb8"], xb8, None, k2, d, rc, bias_sb["bkb"])
                 for rc in range(2) for d in range(4)] +
                [lambda d=d, rc=rc: proj_qk(W["wqb8"], xb8, None, q2, d, rc, bias_sb["bqb"])
                 for rc in range(2) for d in range(4)] +
                [mk_v(W["wvb8"], W["wvbr8"], xb8, xbr8, v2, kb, 0, 1) for kb in range(4)] +
                [lambda d=d: out_proj(W["wo16"], ao_t, ya, 0, 6, bias_sb["boa"], d)
                 for d in range(6)]
            )
            attention(q_t, k_t, v_t, ao_t, bias_sb["bva"], NH, hooksA)

            hooksB = {
                0: [mk_v(W["wvb8"], W["wvbr8"], xb8, xbr8, v2, kb, 0, 1)
                    for kb in range(4, NKB)],
                1: [lambda d=d: out_proj(W["wo16"], ao_t, ya, 1, 6, bias_sb["boa"], d)
                    for d in range(6)] +
                   [lambda d=d: out_proj(W["wob16"], ao2, yb, 0, 3, None, d)
                    for d in range(6)],
            }
            attention(q2, k2, v2, ao2, bias_sb["bvb"], 6, hooksB)
            for d in range(6):
                out_proj(W["wob16"], ao2, yb, 1, 3, None, d, mk=mk_rr_psum)

    nc.compile()
    return nc


_NC = None


def _get_nc():
    global _NC
    if _NC is None:
        _NC = build_nc()
    return _NC


def _q8(a, scale):
    return np.asarray(np.clip(a * scale, -240.0, 240.0), E4)


def _q8r(a, scale):
    """(hi, resid) e4m3 pair in the same scale domain."""
    s = np.clip(a * scale, -240.0, 240.0).astype(np.float32)
    hi = np.asarray(s, E4)
    r = np.asarray(s - hi.astype(np.float32), E4)
    return hi, r


def _pairpack(m):
    """[768, cols] -> [128, 3, 2, cols] (cb pairs along member axis)."""
    c, cols = m.shape
    return np.ascontiguousarray(m.reshape(3, 2, 128, cols).transpose(2, 0, 1, 3))


def _qk_perm(nh):
    """Column permutation producing triad layout: dblk 2j = heads 3j..3j+2
    dims 0-31 (+32 pad), dblk 2j+1 = dims 32-63 (+32 pad)."""
    idx, pad = [], []
    ntri = (nh + 2) // 3
    for j in range(ntri):
        for mhalf in range(2):
            for i in range(4):
                h = 3 * j + i
                if i < 3 and h < nh:
                    idx.extend(range(h * HD + 32 * mhalf, h * HD + 32 * mhalf + 32))
                    pad.extend([False] * 32)
                else:
                    idx.extend([0] * 32)  # pad slot
                    pad.extend([True] * 32)
    return np.array(idx), np.array(pad)


def make_in_maps(x, Wq, bq, Wk, bk, Wv, bv, Wo, bo):
    f = np.float32
    permA, padA = _qk_perm(12)
    permB6, padB6 = _qk_perm(6)  # within the 6-head slice

    def qk_pack(Wfull, perm, pad):
        Wt = Wfull.T.astype(f)  # [c, d]
        Wp = Wt[:, perm] * (~pad)[None, :]
        return _pairpack(_q8(Wp, S_W))

    def bias_qk(b, perm, pad, ndblk):
        bp = b[perm].astype(f) * S_QK * (~pad)
        return np.ascontiguousarray(bp.reshape(ndblk, 128).T)

    wq8 = qk_pack(Wq, permA, padA)
    wk8 = qk_pack(Wk, permA, padA)
    wv_hi, wv_r = _q8r(Wv.T.astype(f), S_W)
    wv8, wvr8 = _pairpack(wv_hi), _pairpack(wv_r)
    wo16 = np.ascontiguousarray(
        Wo.T.astype(ml_dtypes.bfloat16).reshape(6, 128, 768).transpose(1, 0, 2))
    bqa = bias_qk(bq, permA, padA, 8)
    bka = bias_qk(bk, permA, padA, 8)
    bva = np.ascontiguousarray(bv.astype(f).reshape(6, 128).T)
    boa = np.ascontiguousarray(bo.astype(f).reshape(6, 128).T)
    ident = np.eye(128, dtype=ml_dtypes.bfloat16)

    in_maps = []
    for c in range(N_CORES):
        par, bb = c % 2, 8 + c // 2
        hsl = slice(par * 6 * HD, (par * 6 + 6) * HD)
        xa_hi, xa_r = _q8r(x[c].T.astype(f), S_X)
        xb_hi, xb_r = _q8r(x[bb].T.astype(f), S_X)
        WqB, WkB, WvB = Wq[hsl], Wk[hsl], Wv[hsl]
        wvb_hi, wvb_r = _q8r(WvB.T.astype(f), S_W)
        wob16 = np.ascontiguousarray(
            Wo[:, hsl].T.astype(ml_dtypes.bfloat16).reshape(3, 128, 768).transpose(1, 0, 2))
        in_maps.append({
            "xa8": _pairpack(xa_hi), "xar8": _pairpack(xa_r),
            "xb8": _pairpack(xb_hi), "xbr8": _pairpack(xb_r),
            "wq8": wq8, "wk8": wk8, "wv8": wv8, "wvr8": wvr8, "wo16": wo16,
            "wqb8": qk_pack(WqB, permB6, padB6),
            "wkb8": qk_pack(WkB, permB6, padB6),
            "wvb8": _pairpack(wvb_hi), "wvbr8": _pairpack(wvb_r),
            "wob16": wob16,
            "bqa": bqa, "bka": bka,
            "bqb": bias_qk(bq[hsl], permB6, padB6, 4),
            "bkb": bias_qk(bk[hsl], permB6, padB6, 4),
            "bva": bva,
            "bvb": np.ascontiguousarray(bv[hsl].astype(f).reshape(3, 128).T),
            "boa": boa,
            "ident": ident,
        })
    return in_maps


def assemble(results, bo):
    out = np.empty((B, T, C), np.float32)
    for c in range(8):
        out[c] = results[c]["ya_t"].T
    for j in range(4):
        out[8 + j] = (results[2 * j]["yb_t"].astype(np.float32)
                      + results[2 * j + 1]["yb_t"].astype(np.float32)).T + bo
    return out


def kernel(**inputs):
    nc = _get_nc()
    in_maps = make_in_maps(**inputs)
    res = run_bass_kernel_spmd(nc, in_maps, list(range(N_CORES)))
    return assemble(res.results, inputs["bo"].astype(np.float32))


if __name__ == "__main__":
    rng = np.random.default_rng(0)
    inputs = {
        "x": rng.normal(size=(B, T, C)).astype(np.float32),
        **{k: (rng.normal(size=(C, C)) * 0.02).astype(np.float32)
           for k in ("Wq", "Wk", "Wv", "Wo")},
        **{k: (rng.normal(size=(C,)) * 0.02).astype(np.float32)
           for k in ("bq", "bk", "bv", "bo")},
    }
    out = kernel(**inputs)
    print(out.shape, out.dtype)
